# revision 21
# baseline (speedup 1.0000x reference)
"""Trainium2 Bass kernel for nn_FM_LOSS_12146167513244.

loss = mean((selfattn(f_s) - BN(W @ f_t))^2)   with b=8, c=512, n=2048, h=8, d=64.

Data-parallel over batch: one batch element per core, 8 cores. BatchNorm uses
batch-global statistics, so the loss is expanded algebraically and each core
only produces per-channel partial sums over its local n columns:

  A_o = sum_n x^2          B_o = sum_n x          (x = W @ f_t, pre-BN)
  E_o = sum_n fs^2         D_o = sum_n fs         C_o = sum_n fs * x

(stored as bn_stats mean/var pairs plus the raw C sum). The host reduces
across cores in float64 and closes the formula:

  femb = s_o * x + t_o,  s_o = gamma/sqrt(var+eps), t_o = beta - mean*s_o
  SSE  = sum_o [E_o - 2(s_o C_o + t_o D_o) + s_o^2 A_o + 2 s_o t_o B_o + Nt t_o^2]

On-core attention (q = k = v = f_s head slice Fh [64, n]):
  S = Fh^T Fh, tiles produced in [j, i] layout (j = softmax reduction index on
  partitions) so the P @ V contraction needs no transpose of P. Overflow-safe
  softmax without a max pass: subtract the per-column Cauchy-Schwarz bound
  Mhat_i = sqrt(nu_i) * sqrt(max_k nu_k) >= max_j q_i.q_j  (nu = |q|^2).
  The bound's slack vs the true column max is at most max_k nu_k / 4 (~56 for
  this data), so the column's dominant exp term stays a normal fp32 number;
  terms below exp(-103) flush to zero harmlessly. (The AM-GM bound
  (nu_i + nu_max)/2 is NOT safe: its slack reaches (nu_max - nu_i)/2 ~ 92,
  the whole column flushes, and 1/l = inf -> NaN.)
  The subtraction rides in the S matmul as a rank-1 K-augmentation (ones row
  in lhsT x -Mhat row in rhs, K=65); the softmax denominator rides in the
  P @ V matmul as an extra ones column (M=65).
"""

import numpy as np

C = 512
N = 2048
H = 8
D = 64
NCORES = 8
BN_EPS = 1e-5

NCT = C // 128   # 4 channel tiles
NJT = N // 128   # 16 j-chunks
NIS = N // 512   # 4 i-strips
JPR = 2          # j-chunks per exp round
NR = NJT // JPR  # 8 rounds per strip


def build_nc():
    import concourse.bass as bass
    import concourse.bacc as bacc
    import concourse.tile as tile
    from concourse import mybir
    from concourse.masks import make_identity
    from contextlib import ExitStack

    fp32 = mybir.dt.float32
    dmm = mybir.dt.float32r
    AF = mybir.ActivationFunctionType
    ALU = mybir.AluOpType
    AX = mybir.AxisListType

    nc = bacc.Bacc(None, target_bir_lowering=False)
    fs_d = nc.dram_tensor("f_s", [C, N], fp32, kind="ExternalInput")
    ft_d = nc.dram_tensor("f_t", [C, N], fp32, kind="ExternalInput")
    w_d = nc.dram_tensor("W", [C, C], fp32, kind="ExternalInput")
    st_d = nc.dram_tensor("stats", [C, 5], fp32, kind="ExternalOutput")

    with tile.TileContext(nc) as tc, ExitStack() as ctx:
        persist = ctx.enter_context(tc.tile_pool(name="persist", bufs=1))

        ident = persist.tile([128, 128], dmm, tag="ident")
        ones_n = persist.tile([1, N], dmm, tag="onesn")
        ones32_f = persist.tile([128, 32], fp32, tag="ones32f")
        nc.vector.memset(ones32_f, 1.0)
        # per-tile head indicator columns for the nu (column-sum) matmuls
        ind = persist.tile([128, 4 * 8], dmm, tag="ind")

        mhat_all = persist.tile([8, N], fp32, tag="mhatall")

        fs_sb = [persist.tile([128, N], dmm, tag=f"fs{t}", name=f"fs{t}")
                 for t in range(NCT)]
        ft_sb = [persist.tile([128, N], dmm, tag=f"ft{t}", name=f"ft{t}")
                 for t in range(NCT)]
        wt_sb = [persist.tile([128, C], dmm, tag=f"wt{k}", name=f"wt{k}")
                 for k in range(NCT)]
        for t in range(NCT):
            nc.sync.dma_start(
                out=fs_sb[t], in_=fs_d[t * 128:(t + 1) * 128, :].bitcast(dmm))
            nc.sync.dma_start(
                out=ft_sb[t], in_=ft_d[t * 128:(t + 1) * 128, :].bitcast(dmm))

        # ---- prep: W^T tiles, nu -> sqrt -> Mhat rows for all heads ----
        with tc.tile_pool(name="prep", bufs=1) as prep, \
             tc.tile_pool(name="prep_ps", bufs=2, space="PSUM") as prep_ps:
            rtnu = prep.tile([8, N], fp32, tag="rtnu")
            ident_f = prep.tile([128, 128], fp32, tag="identf")
            make_identity(nc, ident_f)
            nc.vector.tensor_copy(ident, ident_f)
            onesn_f = prep.tile([1, N], fp32, tag="onesnf")
            nc.vector.memset(onesn_f, 1.0)
            nc.vector.tensor_copy(ones_n, onesn_f)
            ind_f = prep.tile([128, 4 * 8], fp32, tag="indf")
            nc.vector.memset(ind_f, 0.0)
            for t in range(NCT):
                nc.vector.memset(ind_f[0:64, t * 8 + 2 * t:t * 8 + 2 * t + 1], 1.0)
                nc.vector.memset(
                    ind_f[64:128, t * 8 + 2 * t + 1:t * 8 + 2 * t + 2], 1.0)
            nc.vector.tensor_copy(ind, ind_f)
            w_sb = [prep.tile([128, C], dmm, tag=f"w{t}", name=f"w{t}")
                    for t in range(NCT)]
            for t in range(NCT):
                nc.sync.dma_start(
                    out=w_sb[t], in_=w_d[t * 128:(t + 1) * 128, :].bitcast(dmm))
            for t in range(NCT):        # o tile (row block of W)
                for t2 in range(NCT):   # c tile (col block of W)
                    pt = prep_ps.tile([128, 128], dmm, tag="wtp")
                    nc.tensor.transpose(
                        pt, w_sb[t][:, t2 * 128:(t2 + 1) * 128], ident)
                    nc.scalar.copy(
                        wt_sb[t2][:, t * 128:(t + 1) * 128], pt)

            fsq = [prep.tile([128, N], dmm, tag=f"fsq{t}", name=f"fsq{t}")
                   for t in range(NCT)]
            for t in range(NCT):
                nc.vector.tensor_mul(fsq[t], fs_sb[t], fs_sb[t])
            for s in range(NIS):
                nups = prep_ps.tile([8, 512], fp32, tag="nups")
                for t in range(NCT):
                    nc.tensor.matmul(
                        nups, ind[:, t * 8:(t + 1) * 8],
                        fsq[t][:, s * 512:(s + 1) * 512],
                        start=(t == 0), stop=(t == NCT - 1))
                nc.scalar.activation(
                    rtnu[:, s * 512:(s + 1) * 512], nups, AF.Sqrt)
            rtmax = prep.tile([8, 1], fp32, tag="rtmax")
            nc.vector.reduce_max(rtmax, rtnu, axis=AX.X)
            nrtmax = prep.tile([8, 1], fp32, tag="nrtmax")
            nc.vector.tensor_scalar(
                out=nrtmax, in0=rtmax, scalar1=-1.0, scalar2=None,
                op0=ALU.mult)
            # Per-head exponent shift: scales numerator and denominator of the
            # softmax by e^shift (cancels in fs), lifting the dominant
            # denominator term clear of the hw exp flush (~e^-94) when the
            # C-S slack (<= numax/4) is large. shift = clamp(numax/4 - 64, 0, 79):
            # no exp overflow (<= 79 < 88), flushed terms stay >= 30 below the
            # dominant one.
            numax8 = prep.tile([8, 1], fp32, tag="numax8")
            nc.vector.tensor_mul(numax8, rtmax, rtmax)
            shift8 = prep.tile([8, 1], fp32, tag="shift8")
            nc.vector.tensor_scalar(
                out=shift8, in0=numax8, scalar1=0.25, scalar2=-64.0,
                op0=ALU.mult, op1=ALU.add)
            nc.vector.tensor_scalar_max(shift8, shift8, 0.0)
            nc.vector.tensor_scalar_min(shift8, shift8, 79.0)
            # mhat_all[h, i] = -sqrt(nu_i) * sqrt(max nu) + shift_h
            nc.scalar.activation(
                mhat_all, rtnu, AF.Copy, scale=nrtmax)
            nc.vector.tensor_scalar(
                out=mhat_all, in0=mhat_all, scalar1=shift8, scalar2=None,
                op0=ALU.add)

        # ---- attention + conv, pipelined over heads ----
        with tc.tile_pool(name="aug", bufs=2) as augp, \
             tc.tile_pool(name="fstp", bufs=2) as fstp, \
             tc.tile_pool(name="xp", bufs=2) as xp, \
             tc.tile_pool(name="pp", bufs=3) as pp, \
             tc.tile_pool(name="sm", bufs=2) as sm, \
             tc.tile_pool(name="s_ps", bufs=2, space="PSUM") as s_ps, \
             tc.tile_pool(name="o_ps", bufs=2, space="PSUM") as o_ps, \
             tc.tile_pool(name="aux_ps", bufs=2, space="PSUM") as aux_ps:
            pair_tiles = {}

            def pair_ops(tp):
                """Closures issuing head-pair tp's conv x and [V^T|ones]
                tiles in PE-sized chunks, used as filler between attention
                rounds to keep the PE from idling (and its clock ramped)."""
                xpair = xp.tile([128, N], fp32, tag="xpair")
                fsta = fstp.tile([128, NJT * 130], dmm, tag="fsta")
                pair_tiles[tp] = (xpair, fsta)
                ops = [lambda: nc.vector.tensor_copy(fsta[:, 64::65], ones32_f)]

                def conv_strip(s):
                    pc = aux_ps.tile([128, 512], fp32, tag="aux", name="pc")
                    for k in range(NCT):
                        nc.tensor.matmul(
                            pc, wt_sb[k][:, tp * 128:(tp + 1) * 128],
                            ft_sb[k][:, s * 512:(s + 1) * 512],
                            start=(k == 0), stop=(k == NCT - 1))
                    nc.vector.tensor_copy(xpair[:, s * 512:(s + 1) * 512], pc)

                def tp2(j0):
                    for j in (j0, j0 + 1):
                        pt = aux_ps.tile([128, 128], dmm, tag="aux", name="tp")
                        nc.tensor.transpose(
                            pt, fs_sb[tp][:, j * 128:(j + 1) * 128], ident)
                        nc.vector.tensor_copy(
                            fsta[:, j * 130:j * 130 + 64], pt[:, 0:64])
                        nc.vector.tensor_copy(
                            fsta[:, j * 130 + 65:j * 130 + 129], pt[:, 64:128])

                def xstats():
                    x6 = sm.tile([128, NIS, 6], fp32, tag="x6")
                    for s in range(NIS):
                        nc.vector.bn_stats(
                            x6[:, s, :], xpair[:, s * 512:(s + 1) * 512])
                    mvx = sm.tile([128, 2], fp32, tag="mvx")
                    nc.vector.bn_aggr(mvx, x6)
                    nc.sync.dma_start(
                        out=st_d[tp * 128:(tp + 1) * 128, 0:2], in_=mvx)

                for s in range(NIS):
                    ops.append(lambda s=s: conv_strip(s))
                for j0 in range(0, NJT, 2):
                    ops.append(lambda j0=j0: tp2(j0))
                ops.append(xstats)
                return ops

            def issue_augs(h):
                th, q0 = h // 2, (h % 2) * 64
                fh = fs_sb[th][q0:q0 + 64, :]
                a1 = augp.tile([65, N], dmm, tag="aug1")
                a2 = augp.tile([65, N], dmm, tag="aug2")
                nc.sync.dma_start(out=a1[0:64, :], in_=fh)
                nc.sync.dma_start(out=a1[64:65, :], in_=ones_n)
                nc.sync.dma_start(out=a2[0:64, :], in_=fh)
                nc.sync.dma_start(
                    out=a2[64:65, :], in_=mhat_all[h:h + 1, :].bitcast(dmm))
                return a1, a2

            for op in pair_ops(0):
                op()
            augs = {0: issue_augs(0)}
            fill = []

            for hh in range(H):
                t, p0 = hh // 2, (hh % 2) * 64
                xpair, fsta = pair_tiles[t]
                aug1, aug2 = augs.pop(hh)
                if hh + 1 < H:
                    augs[hh + 1] = issue_augs(hh + 1)
                if hh % 2 == 1 and t + 1 < NCT:
                    fill = pair_ops(t + 1)

                f6 = sm.tile([128, NIS, 6], fp32, tag="f6")
                cpart = sm.tile([128, NIS], fp32, tag="cpart")
                for s in range(NIS):
                    po = o_ps.tile([65, 512], fp32, tag="po")
                    ptils = []
                    for r in range(NR):
                        if fill and r % 2 == 0:
                            fill.pop(0)()
                        ps = s_ps.tile([128, JPR * 512], fp32, tag="ps")
                        for k in range(JPR):
                            j = r * JPR + k
                            nc.tensor.matmul(
                                ps[:, k * 512:(k + 1) * 512],
                                aug1[:, j * 128:(j + 1) * 128],
                                aug2[:, s * 512:(s + 1) * 512],
                                start=True, stop=True)
                        ptil = pp.tile([128, JPR * 512], dmm, tag="ptil")
                        nc.scalar.activation(ptil, ps, AF.Exp)
                        ptils.append(ptil)
                        # issue P@V for the PREVIOUS round so the PE can run
                        # this round's S matmuls while ACT exps this round
                        if r >= 1:
                            for k in range(JPR):
                                j = (r - 1) * JPR + k
                                b0 = j * 130 + (p0 // 64) * 65
                                nc.tensor.matmul(
                                    po, fsta[:, b0:b0 + 65],
                                    ptils[r - 1][:, k * 512:(k + 1) * 512],
                                    start=(j == 0), stop=(j == NJT - 1))
                    for k in range(JPR):
                        j = (NR - 1) * JPR + k
                        b0 = j * 130 + (p0 // 64) * 65
                        nc.tensor.matmul(
                            po, fsta[:, b0:b0 + 65],
                            ptils[NR - 1][:, k * 512:(k + 1) * 512],
                            start=(j == 0), stop=(j == NJT - 1))

                    lrow = sm.tile([1, 512], fp32, tag="lrow")
                    nc.vector.tensor_copy(lrow, po[64:65, :])
                    rrow = sm.tile([1, 512], fp32, tag="rrow")
                    nc.vector.reciprocal_approx_fast(out=rrow, in_=lrow)
                    r64 = sm.tile([128, 512], fp32, tag="r64")
                    nc.gpsimd.partition_broadcast(r64, rrow)
                    fs_t = sm.tile([128, 512], fp32, tag="fst")
                    nc.vector.tensor_mul(
                        fs_t[p0:p0 + 64, :], po[0:64, :], r64[p0:p0 + 64, :])
                    nc.vector.bn_stats(f6[p0:p0 + 64, s, :], fs_t[p0:p0 + 64, :])
                    scr = sm.tile([128, 512], fp32, tag="scr", bufs=1)
                    nc.gpsimd.tensor_mul(
                        scr[p0:p0 + 64, :], fs_t[p0:p0 + 64, :],
                        xpair[p0:p0 + 64, s * 512:(s + 1) * 512])
                    nc.vector.reduce_sum(
                        cpart[p0:p0 + 64, s:s + 1], scr[p0:p0 + 64, :], axis=AX.X)
                for op in fill:
                    op()
                fill = []
                mvf = sm.tile([128, 2], fp32, tag="mvf")
                nc.vector.bn_aggr(mvf[p0:p0 + 64, :], f6[p0:p0 + 64, :, :])
                cacc = sm.tile([128, 1], fp32, tag="cacc")
                nc.vector.reduce_sum(
                    cacc[p0:p0 + 64, :], cpart[p0:p0 + 64, :], axis=AX.X)
                nc.sync.dma_start(
                    out=st_d[hh * 64:(hh + 1) * 64, 2:4], in_=mvf[p0:p0 + 64, :])
                nc.sync.dma_start(
                    out=st_d[hh * 64:(hh + 1) * 64, 4:5], in_=cacc[p0:p0 + 64, :])
    nc.compile()
    return nc


def combine_stats(stats, gamma, beta, n=N):
    """stats: [m_cores, C, 5] per-core per-channel (x_mean, x_var, fs_mean,
    fs_var, C_raw) over the core's local n columns. Returns fp32 loss."""
    st = np.asarray(stats, dtype=np.float64)
    m = st.shape[0]
    nt = float(m * n)
    A = (st[:, :, 1] + st[:, :, 0] ** 2).sum(0) * n
    B = st[:, :, 0].sum(0) * n
    E = (st[:, :, 3] + st[:, :, 2] ** 2).sum(0) * n
    Dm = st[:, :, 2].sum(0) * n
    Cs = st[:, :, 4].sum(0)
    mean = B / nt
    var = A / nt - mean ** 2
    s = np.asarray(gamma, np.float64) / np.sqrt(var + BN_EPS)
    tt = np.asarray(beta, np.float64) - mean * s
    sse = (E - 2.0 * (s * Cs + tt * Dm) + s ** 2 * A + 2.0 * s * tt * B
           + nt * tt ** 2).sum()
    return np.float32(sse / (nt * C))


_CACHE = {}


def kernel(f_s, f_t, W, gamma, beta):
    from concourse.bass_utils import run_bass_kernel_spmd

    if "nc" not in _CACHE:
        _CACHE["nc"] = build_nc()
    nc = _CACHE["nc"]
    f_s = np.ascontiguousarray(f_s, dtype=np.float32)
    f_t = np.ascontiguousarray(f_t, dtype=np.float32)
    W = np.ascontiguousarray(W, dtype=np.float32)
    in_maps = [{"f_s": f_s[i], "f_t": f_t[i], "W": W} for i in range(NCORES)]
    res = run_bass_kernel_spmd(nc, in_maps, list(range(NCORES)))
    _CACHE["last_res"] = res
    stats = np.stack([res.results[i]["stats"] for i in range(NCORES)])
    return np.asarray(combine_stats(stats, gamma, beta), dtype=np.float32)
